# revision 20
# baseline (speedup 1.0000x reference)
"""nn_ChannelAttention Trainium2 Bass kernel (8-core SPMD, data-parallel over batch).

Input  x:   [8, 64, 32, 128, 128] f32
Output att: [8, 64, 1, 1, 1] f32
  n[s]   = sum_c x[c,s]^2
  r[s]   = 1/sqrt(n[s]) = exp(-0.5*ln n)   (channel norms are ~8, eps clamp is a no-op)
  att[c] = sigmoid( relu( mean_s(x*r) + max_s(x*r) )^2 )

Per-core layout: SBUF partitions = (h, c) with h in {0,1} spatial halves,
free = spatial. The v0 kernel was DVE-bound: tensor_mul + tensor_max per
[128,512] tile ~ 2.6us effective (op + drain + dispatch). v5 quarters the
DVE stream: ONE native TENSOR_TENSOR_REDUCE per [128,1024] slab computes
  xn = x * rbp          (fp16 out, feeds the PE spatial-sum accumulation)
  maxbuf[:, si] = max-reduce(xn)   (fp32-exact running max, final reduce_max)
(InstISA ops need mybir.codegen_inst_isa_subclasses() in raw Bass --
without it walrus sees empty .instr bytes and dies with "ISA wrong length".)
  - spatial sum stays on PE (identity-matmul accumulation into PSUM)
  - squares on ACT in [128,2048] ops; rsqrt via Ln+Exp on 16x-redundant
    banded norms (partition redundancy is free: ACT time ~ free size)
"""

from contextlib import ExitStack

import numpy as np

import concourse.bass as bass
import concourse.mybir as mybir
import concourse.tile as tile
import concourse.dve_ops as dve_ops
from concourse.dve_ops import DveOp
from concourse.dve_spec import Spec, Src0, Src1, C0, maxx, lower
from concourse.dve_uop import DveOpSpec

F32 = mybir.dt.float32
FP16 = mybir.dt.float16
AF = mybir.ActivationFunctionType
ALU = mybir.AluOpType


def _ref_tt_mul_max(in0, in1, c0, c1, c2):
    b = (in0.astype(np.float32) * in1.astype(np.float32)).astype(np.float32)
    return b, np.maximum(c0, b.reshape(b.shape[0], -1).max(axis=-1, keepdims=True))


def _register_tt_mul_max():
    """Custom DVE op: out = in0*in1, accum_out = max(s0, max(out)).

    The native TENSOR_TENSOR_REDUCE ISA op compiles but hangs this runtime
    (its uop program isn't in the NRT-loaded DVE tables); the ant custom-DVE
    path ships the uop table per-NEFF, so it actually executes. HW-verified
    exact vs numpy.
    """
    name = "TT_MUL_MAX_ANT"
    if name in dve_ops._SUB_OPCODE_FOR_NAME:
        return next(op for op in dve_ops.OPS if op.name == name)
    spec = Spec(body=Src0 * Src1, accum=maxx, accum_init=C0,
                reference=_ref_tt_mul_max)
    shas = {v: DveOpSpec(name=name, opcode=0, uops=lower(spec, ver=v),
                         rd1_en=True).sha(v) for v in ("v3", "v4")}
    op = DveOp(name, spec, subdim=False, uops_sha=shas)
    dve_ops.OPS.append(op)
    dve_ops._SUB_OPCODE_FOR_NAME[name] = (
        dve_ops._CUSTOM_DVE_ROW_BASE + len(dve_ops.OPS) - 1)
    dve_ops.CUSTOM_DVE_SPECS[name] = spec
    return op


TT_MUL_MAX = _register_tt_mul_max()

B, C, D, H, W = 8, 64, 32, 128, 128
S = D * H * W            # 524288 spatial positions per sample
N_CORES = 8

F = 512                  # PSUM bank width in f32 / matmul dest granularity
SLAB = 1024              # DVE mul + GPSIMD max granularity
QUAD = 2048              # ACT square slab = one nacc (4-band) group
TPG = 16                 # 512-chunks per DMA group (GF = 8192 -> 4 MB loads)


def _build_kernel_body(nc, comp_dt=FP16, repeat=1, use_hwdge=False):
    C_, P = 64, 128
    HALF = S // 2                  # free elements per partition
    GF = F * TPG                   # 8192
    NG = HALF // GF                # 32 DMA groups
    NQ = GF // QUAD                # 4 quads per group
    SPG = GF // SLAB               # 8 slabs per group
    NCHUNK = HALF // F             # 512 chunks total
    assert NG * GF == HALF

    x = nc.dram_tensor("x", [C_, S], F32, kind="ExternalInput")
    y = nc.dram_tensor("att", [C_, 1], F32, kind="ExternalOutput")
    xr = x.ap().rearrange("c (h s) -> h c s", h=2)   # element order (h, c, s)

    ld = nc.sync if use_hwdge else nc.gpsimd

    with tile.TileContext(nc) as tc, ExitStack() as ctx:
        const_pool = ctx.enter_context(tc.tile_pool(name="const", bufs=1))
        gbuf_pool = ctx.enter_context(tc.tile_pool(name="gbuf", bufs=2))
        sq_pool = ctx.enter_context(tc.tile_pool(name="sq", bufs=3))
        nacc_pool = ctx.enter_context(tc.tile_pool(name="nacc", bufs=2, space="PSUM"))
        rbp_pool = ctx.enter_context(tc.tile_pool(name="rbp", bufs=2, space="PSUM"))
        sacc_pool = ctx.enter_context(tc.tile_pool(name="sacc", bufs=1, space="PSUM"))
        lall_pool = ctx.enter_context(tc.tile_pool(name="lall", bufs=3))
        xn_pool = ctx.enter_context(tc.tile_pool(name="xn", bufs=6))
        acc_pool = ctx.enter_context(tc.tile_pool(name="acc", bufs=1))
        fin_pool = ctx.enter_context(tc.tile_pool(name="fin", bufs=1))

        # lhsT32[p, m] = 1 iff p//64 == m%2  (out row m = n[h=m%2])
        lhsT32 = const_pool.tile([P, 32], comp_dt)
        nc.vector.memset(lhsT32[:], 0.0)
        lo = lhsT32[0:64, :].rearrange("p (m two) -> p m two", two=2)
        nc.vector.memset(lo[:, :, 0:1], 1.0)
        hi = lhsT32[64:128, :].rearrange("p (m two) -> p m two", two=2)
        nc.vector.memset(hi[:, :, 1:2], 1.0)
        # sel2_all rows 32b+h: ones at cols h*64:(h+1)*64 (placed by DMA --
        # engine ops cannot start at partition 32b+1)
        sel2_all = const_pool.tile([P, P], comp_dt)
        nc.vector.memset(sel2_all[:], 0.0)
        rowpat = const_pool.tile([1, 2 * P], comp_dt)
        nc.vector.memset(rowpat[:], 0.0)
        nc.vector.memset(rowpat[0:1, 0:64], 1.0)
        nc.vector.memset(rowpat[0:1, 192:256], 1.0)
        for b4 in range(4):
            nc.gpsimd.dma_start(sel2_all[32 * b4:32 * b4 + 2, :], rowpat[0:1, :])

        ident = const_pool.tile([P, P], comp_dt)
        ones_t = const_pool.tile([P, P], comp_dt)
        nc.vector.memset(ones_t[:], 1.0)
        nc.gpsimd.affine_select(ident[:], ones_t[:], pattern=[[1, P]], base=0,
                                channel_multiplier=-1, compare_op=ALU.is_equal,
                                fill=0.0)

        sacc = sacc_pool.tile([P, F], F32)
        NSLAB = HALF // SLAB
        maxbuf = acc_pool.tile([P, NSLAB * repeat], F32)

        n_chunks_total = repeat * NCHUNK
        for rep in range(repeat):         # >1 only for timing builds
            for g in range(NG):
                gbuf = gbuf_pool.tile([P, GF], F32)
                ld.dma_start(gbuf[:], xr[:, :, g * GF:(g + 1) * GF])

                for q in range(NQ):
                    qi = (rep * NG + g) * NQ + q     # global quad idx
                    qbase = q * QUAD
                    sq = sq_pool.tile([P, QUAD], comp_dt, tag="sq")
                    nc.scalar.square(sq[:], gbuf[:, qbase:qbase + QUAD])
                    nacc = nacc_pool.tile([P, F], F32)
                    for b in range(4):
                        nc.tensor.matmul(
                            nacc[32 * b:32 * b + 32, :],
                            lhsT32[:], sq[:, b * F:(b + 1) * F],
                            start=True, stop=True, tile_position=(0, 32 * b))

                    l_all = lall_pool.tile([P, F], F32, tag="lall")
                    nc.scalar.activation(l_all[:], nacc[:], AF.Ln)
                    r_all = lall_pool.tile([P, F], comp_dt, tag="rall")
                    nc.scalar.activation(r_all[:], l_all[:], AF.Exp, scale=-0.5)

                    for h2 in range(2):
                        rbp = rbp_pool.tile([P, SLAB], F32)
                        for j in range(2):
                            b = 2 * h2 + j
                            nc.tensor.matmul(
                                rbp[:, j * F:(j + 1) * F],
                                sel2_all[32 * b:32 * b + 2, :],
                                r_all[32 * b:32 * b + 2, :],
                                start=True, stop=True, tile_position=(32 * b, 0))

                        si = (rep * NG + g) * SPG + q * 2 + h2   # global slab idx
                        xn = xn_pool.tile([P, SLAB], comp_dt, tag="xn")
                        nc.vector._custom_dve(
                            TT_MUL_MAX, out=xn[:],
                            in0=gbuf[:, qbase + h2 * SLAB:qbase + (h2 + 1) * SLAB],
                            in1=rbp[:], s0=-3.0e38,
                            accum_out=maxbuf[:, si:si + 1])

                        for j in range(2):
                            cg = si * 2 + j    # global chunk idx
                            nc.tensor.matmul(
                                sacc[:], ident[:], xn[:, j * F:(j + 1) * F],
                                start=(cg == 0), stop=(cg == n_chunks_total - 1),
                                skip_group_check=True)

        # ---- finalize ----
        sum_pc = fin_pool.tile([P, 1], F32)
        s_sb = fin_pool.tile([P, F], F32)
        nc.scalar.activation(s_sb[:], sacc[:], AF.Copy)
        nc.vector.reduce_sum(sum_pc[:], s_sb[:], axis=mybir.AxisListType.X)
        max_pc = fin_pool.tile([P, 1], F32)
        nc.vector.reduce_max(max_pc[:], maxbuf[:], axis=mybir.AxisListType.X)

        # fold halves (partitions 64:128 -> 0:64) via SBUF->SBUF DMA realign
        hi2 = fin_pool.tile([64, 2], F32)
        nc.gpsimd.dma_start(hi2[:, 0:1], sum_pc[64:128, :])
        nc.gpsimd.dma_start(hi2[:, 1:2], max_pc[64:128, :])
        s64 = fin_pool.tile([64, 1], F32)
        nc.vector.tensor_add(s64[:], sum_pc[0:64, :], hi2[:, 0:1])
        m64 = fin_pool.tile([64, 1], F32)
        nc.vector.tensor_max(m64[:], max_pc[0:64, :], hi2[:, 1:2])
        avg = fin_pool.tile([64, 1], F32)
        nc.vector.tensor_scalar_mul(avg[:], s64[:], 1.0 / (S * repeat))
        o = fin_pool.tile([64, 1], F32)
        nc.vector.tensor_add(o[:], avg[:], m64[:])
        orelu = fin_pool.tile([64, 1], F32)
        nc.vector.tensor_scalar_max(orelu[:], o[:], 0.0)
        o2 = fin_pool.tile([64, 1], F32)
        nc.vector.tensor_mul(o2[:], orelu[:], orelu[:])
        att_s = fin_pool.tile([64, 1], F32)
        nc.scalar.activation(att_s[:], o2[:], AF.Sigmoid)
        nc.gpsimd.dma_start(y.ap(), att_s[:])
    return nc


def _split_multi_waits(nc, max_waits=1):
    """This walrus build encodes at most one sync-wait per CTRL instruction;
    hoist extra waits into single-wait NoOps placed just before."""
    for f in nc.m.functions:
        for bb in f.blocks:
            insts = list(bb.instructions)
            out = []
            changed = False
            for ins in insts:
                si = ins.sync_info
                if si is not None and si.on_wait and len(si.on_wait) > max_waits:
                    waits = list(si.on_wait)
                    for w in waits[:-max_waits]:
                        out.append(mybir.InstNoOp(
                            name=nc.get_next_instruction_name(),
                            sync_info=mybir.SyncInfo(on_wait=[w], on_update=[]),
                            bass_nofuse=True,
                            engine=ins.engine,
                        ))
                    si.on_wait = waits[-max_waits:]
                    ins.sync_info = si
                    changed = True
                out.append(ins)
            if changed:
                bb.instructions = out


def build_nc(repeat=1, **kw):
    nc = bass.Bass("TRN2", target_bir_lowering=False, debug=False,
                   num_devices=N_CORES)
    _build_kernel_body(nc, repeat=repeat, **kw)
    _split_multi_waits(nc)
    # Raw Bass skips the extended-inst lowering pass; without it InstISA
    # instructions (tensor_tensor_reduce) reach walrus with empty .instr.
    mybir.codegen_inst_isa_subclasses(nc)
    return nc


def kernel(x):
    """x: [8, 64, 32, 128, 128] f32 -> att [8, 64, 1, 1, 1] f32."""
    from concourse.bass_utils import run_bass_kernel_spmd

    x = np.ascontiguousarray(np.asarray(x, dtype=np.float32))
    assert x.shape == (B, C, D, H, W)
    nc = build_nc()
    in_maps = [{"x": x[i].reshape(C, S)} for i in range(N_CORES)]
    res = run_bass_kernel_spmd(nc, in_maps, core_ids=list(range(N_CORES)))
    att = np.stack([res.results[i]["att"].reshape(C) for i in range(N_CORES)])
    return att.reshape(B, C, 1, 1, 1).astype(np.float32)


# revision 24
# speedup vs baseline: 1.4753x; 1.4753x over previous
"""nn_ChannelAttention Trainium2 Bass kernel (8-core SPMD, data-parallel over batch).

Input  x:   [8, 64, 32, 128, 128] f32
Output att: [8, 64, 1, 1, 1] f32
  n[s]   = sum_c x[c,s]^2
  r[s]   = 1/sqrt(n[s]) = exp(-0.5*ln n)   (channel norms are ~8, eps clamp is a no-op)
  att[c] = sigmoid( relu( mean_s(x*r) + max_s(x*r) )^2 )

Per-core layout: SBUF partitions = (h, c) with h in {0,1} spatial halves,
free = spatial. The v0 kernel was DVE-bound: tensor_mul + tensor_max per
[128,512] tile ~ 2.6us effective (op + drain + dispatch). v5 quarters the
DVE stream: ONE native TENSOR_TENSOR_REDUCE per [128,1024] slab computes
  xn = x * rbp          (fp16 out, feeds the PE spatial-sum accumulation)
  maxbuf[:, si] = max-reduce(xn)   (fp32-exact running max, final reduce_max)
(InstISA ops need mybir.codegen_inst_isa_subclasses() in raw Bass --
without it walrus sees empty .instr bytes and dies with "ISA wrong length".)
  - spatial sum stays on PE (identity-matmul accumulation into PSUM)
  - squares on ACT in [128,2048] ops; rsqrt via Ln+Exp on 16x-redundant
    banded norms (partition redundancy is free: ACT time ~ free size)
"""

from contextlib import ExitStack

import numpy as np

import concourse.bass as bass
import concourse.mybir as mybir
import concourse.tile as tile
import concourse.dve_ops as dve_ops
from concourse.dve_ops import DveOp
from concourse.dve_spec import Spec, Src0, Src1, C0, maxx, lower
from concourse.dve_uop import DveOpSpec

F32 = mybir.dt.float32
FP16 = mybir.dt.float16
AF = mybir.ActivationFunctionType
ALU = mybir.AluOpType


def _ref_tt_mul_max(in0, in1, c0, c1, c2):
    b = (in0.astype(np.float32) * in1.astype(np.float32)).astype(np.float32)
    return b, np.maximum(c0, b.reshape(b.shape[0], -1).max(axis=-1, keepdims=True))


def _register_tt_mul_max():
    """Custom DVE op: out = in0*in1, accum_out = max(s0, max(out)).

    The native TENSOR_TENSOR_REDUCE ISA op compiles but hangs this runtime
    (its uop program isn't in the NRT-loaded DVE tables); the ant custom-DVE
    path ships the uop table per-NEFF, so it actually executes. HW-verified
    exact vs numpy.
    """
    name = "TT_MUL_MAX_ANT"
    if name in dve_ops._SUB_OPCODE_FOR_NAME:
        return next(op for op in dve_ops.OPS if op.name == name)
    spec = Spec(body=Src0 * Src1, accum=maxx, accum_init=C0,
                reference=_ref_tt_mul_max)
    shas = {v: DveOpSpec(name=name, opcode=0, uops=lower(spec, ver=v),
                         rd1_en=True).sha(v) for v in ("v3", "v4")}
    op = DveOp(name, spec, subdim=False, uops_sha=shas)
    dve_ops.OPS.append(op)
    dve_ops._SUB_OPCODE_FOR_NAME[name] = (
        dve_ops._CUSTOM_DVE_ROW_BASE + len(dve_ops.OPS) - 1)
    dve_ops.CUSTOM_DVE_SPECS[name] = spec
    return op


TT_MUL_MAX = _register_tt_mul_max()

B, C, D, H, W = 8, 64, 32, 128, 128
S = D * H * W            # 524288 spatial positions per sample
N_CORES = 8

F = 512                  # PSUM bank width in f32 / matmul dest granularity
SLAB = 1024              # DVE mul + GPSIMD max granularity
QUAD = 2048              # ACT square slab = one nacc (4-band) group
TPG = 16                 # 512-chunks per DMA group (GF = 8192 -> 4 MB loads)


def _build_kernel_body(nc, comp_dt=FP16, repeat=1, use_hwdge=False, slab=SLAB):
    C_, P = 64, 128
    HALF = S // 2                  # free elements per partition
    GF = F * TPG                   # 8192
    NG = HALF // GF                # 32 DMA groups
    NQ = GF // QUAD                # 4 quads per group
    SPG = GF // slab               # slabs per group
    NCHUNK = HALF // F             # 512 chunks total
    CPS = slab // F                # chunks per slab
    SPQ = QUAD // slab             # slabs per quad
    assert NG * GF == HALF and slab in (512, 1024, 2048)

    x = nc.dram_tensor("x", [C_, S], F32, kind="ExternalInput")
    y = nc.dram_tensor("att", [C_, 1], F32, kind="ExternalOutput")
    xr = x.ap().rearrange("c (h s) -> h c s", h=2)   # element order (h, c, s)

    ld = nc.sync if use_hwdge else nc.gpsimd

    with tile.TileContext(nc) as tc, ExitStack() as ctx:
        const_pool = ctx.enter_context(tc.tile_pool(name="const", bufs=1))
        gbuf_pool = ctx.enter_context(tc.tile_pool(name="gbuf", bufs=2))
        sq_pool = ctx.enter_context(tc.tile_pool(name="sq", bufs=3))
        nacc_pool = ctx.enter_context(tc.tile_pool(name="nacc", bufs=2, space="PSUM"))
        rbp_pool = ctx.enter_context(tc.tile_pool(
            name="rbp", bufs=(2 if slab <= 1024 else 1), space="PSUM"))
        sacc_pool = ctx.enter_context(tc.tile_pool(name="sacc", bufs=1, space="PSUM"))
        lall_pool = ctx.enter_context(tc.tile_pool(name="lall", bufs=3))
        xn_pool = ctx.enter_context(tc.tile_pool(name="xn", bufs=6))
        acc_pool = ctx.enter_context(tc.tile_pool(name="acc", bufs=1))
        fin_pool = ctx.enter_context(tc.tile_pool(name="fin", bufs=1))

        # lhsT32[p, m] = 1 iff p//64 == m%2  (out row m = n[h=m%2])
        lhsT32 = const_pool.tile([P, 32], comp_dt)
        nc.vector.memset(lhsT32[:], 0.0)
        lo = lhsT32[0:64, :].rearrange("p (m two) -> p m two", two=2)
        nc.vector.memset(lo[:, :, 0:1], 1.0)
        hi = lhsT32[64:128, :].rearrange("p (m two) -> p m two", two=2)
        nc.vector.memset(hi[:, :, 1:2], 1.0)
        # sel2_all rows 32b+h: ones at cols h*64:(h+1)*64 (placed by DMA --
        # engine ops cannot start at partition 32b+1)
        sel2_all = const_pool.tile([P, P], comp_dt)
        nc.vector.memset(sel2_all[:], 0.0)
        rowpat = const_pool.tile([1, 2 * P], comp_dt)
        nc.vector.memset(rowpat[:], 0.0)
        nc.vector.memset(rowpat[0:1, 0:64], 1.0)
        nc.vector.memset(rowpat[0:1, 192:256], 1.0)
        for b4 in range(4):
            nc.gpsimd.dma_start(sel2_all[32 * b4:32 * b4 + 2, :], rowpat[0:1, :])

        ident = const_pool.tile([P, P], comp_dt)
        ones_t = const_pool.tile([P, P], comp_dt)
        nc.vector.memset(ones_t[:], 1.0)
        nc.gpsimd.affine_select(ident[:], ones_t[:], pattern=[[1, P]], base=0,
                                channel_multiplier=-1, compare_op=ALU.is_equal,
                                fill=0.0)

        sacc = sacc_pool.tile([P, F], F32)
        NSLAB = HALF // slab
        maxbuf = acc_pool.tile([P, NSLAB * repeat], F32)

        n_chunks_total = repeat * NCHUNK
        for rep in range(repeat):         # >1 only for timing builds
            for g in range(NG):
                gbuf = gbuf_pool.tile([P, GF], F32)
                ld.dma_start(gbuf[:], xr[:, :, g * GF:(g + 1) * GF])

                for q in range(NQ):
                    qi = (rep * NG + g) * NQ + q     # global quad idx
                    qbase = q * QUAD
                    sq = sq_pool.tile([P, QUAD], comp_dt, tag="sq")
                    nc.scalar.square(sq[:], gbuf[:, qbase:qbase + QUAD])
                    nacc = nacc_pool.tile([P, F], F32)
                    for b in range(4):
                        nc.tensor.matmul(
                            nacc[32 * b:32 * b + 32, :],
                            lhsT32[:], sq[:, b * F:(b + 1) * F],
                            start=True, stop=True, tile_position=(0, 32 * b))

                    l_all = lall_pool.tile([P, F], F32, tag="lall")
                    nc.scalar.activation(l_all[:], nacc[:], AF.Ln)
                    r_all = lall_pool.tile([P, F], comp_dt, tag="rall")
                    nc.scalar.activation(r_all[:], l_all[:], AF.Exp, scale=-0.5)

                    for h2 in range(SPQ):
                        rbp = rbp_pool.tile([P, slab], F32)
                        for j in range(CPS):
                            b = CPS * h2 + j
                            nc.tensor.matmul(
                                rbp[:, j * F:(j + 1) * F],
                                sel2_all[32 * b:32 * b + 2, :],
                                r_all[32 * b:32 * b + 2, :],
                                start=True, stop=True, tile_position=(32 * b, 0))

                        si = (rep * NG + g) * SPG + q * SPQ + h2  # global slab idx
                        xn = xn_pool.tile([P, slab], comp_dt, tag="xn")
                        nc.vector._custom_dve(
                            TT_MUL_MAX, out=xn[:],
                            in0=gbuf[:, qbase + h2 * slab:qbase + (h2 + 1) * slab],
                            in1=rbp[:], s0=-3.0e38,
                            accum_out=maxbuf[:, si:si + 1])

                        for j in range(CPS):
                            cg = si * CPS + j    # global chunk idx
                            nc.tensor.matmul(
                                sacc[:], ident[:], xn[:, j * F:(j + 1) * F],
                                start=(cg == 0), stop=(cg == n_chunks_total - 1),
                                skip_group_check=True)

        # ---- finalize ----
        sum_pc = fin_pool.tile([P, 1], F32)
        s_sb = fin_pool.tile([P, F], F32)
        nc.scalar.activation(s_sb[:], sacc[:], AF.Copy)
        nc.vector.reduce_sum(sum_pc[:], s_sb[:], axis=mybir.AxisListType.X)
        max_pc = fin_pool.tile([P, 1], F32)
        nc.vector.reduce_max(max_pc[:], maxbuf[:], axis=mybir.AxisListType.X)

        # fold halves (partitions 64:128 -> 0:64) via SBUF->SBUF DMA realign
        hi2 = fin_pool.tile([64, 2], F32)
        nc.gpsimd.dma_start(hi2[:, 0:1], sum_pc[64:128, :])
        nc.gpsimd.dma_start(hi2[:, 1:2], max_pc[64:128, :])
        s64 = fin_pool.tile([64, 1], F32)
        nc.vector.tensor_add(s64[:], sum_pc[0:64, :], hi2[:, 0:1])
        m64 = fin_pool.tile([64, 1], F32)
        nc.vector.tensor_max(m64[:], max_pc[0:64, :], hi2[:, 1:2])
        avg = fin_pool.tile([64, 1], F32)
        nc.vector.tensor_scalar_mul(avg[:], s64[:], 1.0 / (S * repeat))
        o = fin_pool.tile([64, 1], F32)
        nc.vector.tensor_add(o[:], avg[:], m64[:])
        orelu = fin_pool.tile([64, 1], F32)
        nc.vector.tensor_scalar_max(orelu[:], o[:], 0.0)
        o2 = fin_pool.tile([64, 1], F32)
        nc.vector.tensor_mul(o2[:], orelu[:], orelu[:])
        att_s = fin_pool.tile([64, 1], F32)
        nc.scalar.activation(att_s[:], o2[:], AF.Sigmoid)
        nc.gpsimd.dma_start(y.ap(), att_s[:])
    return nc


def _split_multi_waits(nc, max_waits=1):
    """This walrus build encodes at most one sync-wait per CTRL instruction;
    hoist extra waits into single-wait NoOps placed just before."""
    for f in nc.m.functions:
        for bb in f.blocks:
            insts = list(bb.instructions)
            out = []
            changed = False
            for ins in insts:
                si = ins.sync_info
                if si is not None and si.on_wait and len(si.on_wait) > max_waits:
                    waits = list(si.on_wait)
                    for w in waits[:-max_waits]:
                        out.append(mybir.InstNoOp(
                            name=nc.get_next_instruction_name(),
                            sync_info=mybir.SyncInfo(on_wait=[w], on_update=[]),
                            bass_nofuse=True,
                            engine=ins.engine,
                        ))
                    si.on_wait = waits[-max_waits:]
                    ins.sync_info = si
                    changed = True
                out.append(ins)
            if changed:
                bb.instructions = out


def build_nc(repeat=1, **kw):
    nc = bass.Bass("TRN2", target_bir_lowering=False, debug=False,
                   num_devices=N_CORES)
    _build_kernel_body(nc, repeat=repeat, **kw)
    _split_multi_waits(nc)
    # Raw Bass skips the extended-inst lowering pass; without it InstISA
    # instructions (tensor_tensor_reduce) reach walrus with empty .instr.
    mybir.codegen_inst_isa_subclasses(nc)
    return nc


def kernel(x):
    """x: [8, 64, 32, 128, 128] f32 -> att [8, 64, 1, 1, 1] f32."""
    from concourse.bass_utils import run_bass_kernel_spmd

    x = np.ascontiguousarray(np.asarray(x, dtype=np.float32))
    assert x.shape == (B, C, D, H, W)
    nc = build_nc()
    in_maps = [{"x": x[i].reshape(C, S)} for i in range(N_CORES)]
    res = run_bass_kernel_spmd(nc, in_maps, core_ids=list(range(N_CORES)))
    att = np.stack([res.results[i]["att"].reshape(C) for i in range(N_CORES)])
    return att.reshape(B, C, 1, 1, 1).astype(np.float32)


# revision 29
# speedup vs baseline: 11.4678x; 7.7732x over previous
"""nn_ChannelAttention Trainium2 Bass kernel (8-core SPMD, data-parallel over batch).

Input  x:   [8, 64, 32, 128, 128] f32
Output att: [8, 64, 1, 1, 1] f32
  n[s]   = sum_c x[c,s]^2
  r[s]   = 1/sqrt(n[s]) = exp(-0.5*ln n)   (channel norms are ~8, eps clamp is a no-op)
  att[c] = sigmoid( relu( mean_s(x*r) + max_s(x*r) )^2 )

Per-core layout: SBUF partitions = (h, c) with h in {0,1} spatial halves,
free = spatial. The v0 kernel was DVE-bound: tensor_mul + tensor_max per
[128,512] tile ~ 2.6us effective (op + drain + dispatch). v5 quarters the
DVE stream: ONE native TENSOR_TENSOR_REDUCE per [128,1024] slab computes
  xn = x * rbp          (fp16 out, feeds the PE spatial-sum accumulation)
  maxbuf[:, si] = max-reduce(xn)   (fp32-exact running max, final reduce_max)
(InstISA ops need mybir.codegen_inst_isa_subclasses() in raw Bass --
without it walrus sees empty .instr bytes and dies with "ISA wrong length".)
  - spatial sum stays on PE (identity-matmul accumulation into PSUM)
  - squares on ACT in [128,2048] ops; rsqrt via Ln+Exp on 16x-redundant
    banded norms (partition redundancy is free: ACT time ~ free size)
"""

from contextlib import ExitStack

import numpy as np

import concourse.bass as bass
import concourse.mybir as mybir
import concourse.tile as tile
import concourse.dve_ops as dve_ops
from concourse.dve_ops import DveOp
from concourse.dve_spec import Spec, Src0, Src1, C0, maxx, lower
from concourse.dve_uop import DveOpSpec

F32 = mybir.dt.float32
FP16 = mybir.dt.float16
AF = mybir.ActivationFunctionType
ALU = mybir.AluOpType


def _ref_tt_mul_max(in0, in1, c0, c1, c2):
    b = (in0.astype(np.float32) * in1.astype(np.float32)).astype(np.float32)
    return b, np.maximum(c0, b.reshape(b.shape[0], -1).max(axis=-1, keepdims=True))


def _register_tt_mul_max():
    """Custom DVE op: out = in0*in1, accum_out = max(s0, max(out)).

    The native TENSOR_TENSOR_REDUCE ISA op compiles but hangs this runtime
    (its uop program isn't in the NRT-loaded DVE tables); the ant custom-DVE
    path ships the uop table per-NEFF, so it actually executes. HW-verified
    exact vs numpy.
    """
    name = "TT_MUL_MAX_ANT"
    if name in dve_ops._SUB_OPCODE_FOR_NAME:
        return next(op for op in dve_ops.OPS if op.name == name)
    spec = Spec(body=Src0 * Src1, accum=maxx, accum_init=C0,
                reference=_ref_tt_mul_max)
    shas = {v: DveOpSpec(name=name, opcode=0, uops=lower(spec, ver=v),
                         rd1_en=True).sha(v) for v in ("v3", "v4")}
    op = DveOp(name, spec, subdim=False, uops_sha=shas)
    dve_ops.OPS.append(op)
    dve_ops._SUB_OPCODE_FOR_NAME[name] = (
        dve_ops._CUSTOM_DVE_ROW_BASE + len(dve_ops.OPS) - 1)
    dve_ops.CUSTOM_DVE_SPECS[name] = spec
    return op


TT_MUL_MAX = _register_tt_mul_max()

B, C, D, H, W = 8, 64, 32, 128, 128
S = D * H * W            # 524288 spatial positions per sample
N_CORES = 8

F = 512                  # PSUM bank width in f32 / matmul dest granularity
SLAB = 1024              # DVE mul + GPSIMD max granularity
QUAD = 2048              # ACT square slab = one nacc (4-band) group
TPG = 16                 # 512-chunks per DMA group (GF = 8192 -> 4 MB loads)


def _build_kernel_body(nc, comp_dt=FP16, repeat=1, use_hwdge=False, slab=SLAB,
                       fused=True):
    C_, P = 64, 128
    HALF = S // 2                  # free elements per partition
    GF = F * TPG                   # 8192
    NG = HALF // GF                # 32 DMA groups
    NQ = GF // QUAD                # 4 quads per group
    SPG = GF // slab               # slabs per group
    NCHUNK = HALF // F             # 512 chunks total
    CPS = slab // F                # chunks per slab
    SPQ = QUAD // slab             # slabs per quad
    assert NG * GF == HALF and slab in (512, 1024, 2048)

    x = nc.dram_tensor("x", [C_, S], F32, kind="ExternalInput")
    y = nc.dram_tensor("att", [C_, 1], F32, kind="ExternalOutput")
    xr = x.ap().rearrange("c (h s) -> h c s", h=2)   # element order (h, c, s)

    ld = nc.sync if use_hwdge else nc.gpsimd

    with tile.TileContext(nc) as tc, ExitStack() as ctx:
        const_pool = ctx.enter_context(tc.tile_pool(name="const", bufs=1))
        gbuf_pool = ctx.enter_context(tc.tile_pool(name="gbuf", bufs=2))
        sq_pool = ctx.enter_context(tc.tile_pool(name="sq", bufs=3))
        nacc_pool = ctx.enter_context(tc.tile_pool(name="nacc", bufs=2, space="PSUM"))
        rbp_pool = ctx.enter_context(tc.tile_pool(
            name="rbp", bufs=(2 if slab <= 1024 else 1), space="PSUM"))
        sacc_pool = ctx.enter_context(tc.tile_pool(name="sacc", bufs=1, space="PSUM"))
        lall_pool = ctx.enter_context(tc.tile_pool(name="lall", bufs=3))
        xn_pool = ctx.enter_context(tc.tile_pool(name="xn", bufs=6))
        acc_pool = ctx.enter_context(tc.tile_pool(name="acc", bufs=1))
        fin_pool = ctx.enter_context(tc.tile_pool(name="fin", bufs=1))

        # lhsT32[p, m] = 1 iff p//64 == m%2  (out row m = n[h=m%2])
        lhsT32 = const_pool.tile([P, 32], comp_dt)
        nc.vector.memset(lhsT32[:], 0.0)
        lo = lhsT32[0:64, :].rearrange("p (m two) -> p m two", two=2)
        nc.vector.memset(lo[:, :, 0:1], 1.0)
        hi = lhsT32[64:128, :].rearrange("p (m two) -> p m two", two=2)
        nc.vector.memset(hi[:, :, 1:2], 1.0)
        # sel2_all rows 32b+h: ones at cols h*64:(h+1)*64 (placed by DMA --
        # engine ops cannot start at partition 32b+1)
        sel2_all = const_pool.tile([P, P], comp_dt)
        nc.vector.memset(sel2_all[:], 0.0)
        rowpat = const_pool.tile([1, 2 * P], comp_dt)
        nc.vector.memset(rowpat[:], 0.0)
        nc.vector.memset(rowpat[0:1, 0:64], 1.0)
        nc.vector.memset(rowpat[0:1, 192:256], 1.0)
        for b4 in range(4):
            nc.gpsimd.dma_start(sel2_all[32 * b4:32 * b4 + 2, :], rowpat[0:1, :])

        ident = const_pool.tile([P, P], comp_dt)
        ones_t = const_pool.tile([P, P], comp_dt)
        nc.vector.memset(ones_t[:], 1.0)
        nc.gpsimd.affine_select(ident[:], ones_t[:], pattern=[[1, P]], base=0,
                                channel_multiplier=-1, compare_op=ALU.is_equal,
                                fill=0.0)

        sacc = sacc_pool.tile([P, F], F32)
        NSLAB = HALF // slab
        maxbuf = acc_pool.tile([P, NSLAB * repeat], F32)
        if not fused:
            macc_a = acc_pool.tile([P, slab], comp_dt)
            macc_b = acc_pool.tile([P, slab], comp_dt)
            maccs = [macc_a, macc_b]
            nc.vector.memset(macc_a[:], -2.0)
            nc.vector.memset(macc_b[:], -2.0)

        n_chunks_total = repeat * NCHUNK
        for rep in range(repeat):         # >1 only for timing builds
            for g in range(NG):
                gbuf = gbuf_pool.tile([P, GF], F32)
                ld.dma_start(gbuf[:], xr[:, :, g * GF:(g + 1) * GF])

                for q in range(NQ):
                    qi = (rep * NG + g) * NQ + q     # global quad idx
                    qbase = q * QUAD
                    sq = sq_pool.tile([P, QUAD], comp_dt, tag="sq")
                    nc.scalar.square(sq[:], gbuf[:, qbase:qbase + QUAD])
                    nacc = nacc_pool.tile([P, F], F32)
                    for b in range(4):
                        nc.tensor.matmul(
                            nacc[32 * b:32 * b + 32, :],
                            lhsT32[:], sq[:, b * F:(b + 1) * F],
                            start=True, stop=True, tile_position=(0, 32 * b))

                    l_all = lall_pool.tile([P, F], F32, tag="lall")
                    nc.scalar.activation(l_all[:], nacc[:], AF.Ln)
                    r_all = lall_pool.tile([P, F], comp_dt, tag="rall")
                    nc.scalar.activation(r_all[:], l_all[:], AF.Exp, scale=-0.5)

                    for h2 in range(SPQ):
                        rbp = rbp_pool.tile([P, slab], F32)
                        for j in range(CPS):
                            b = CPS * h2 + j
                            nc.tensor.matmul(
                                rbp[:, j * F:(j + 1) * F],
                                sel2_all[32 * b:32 * b + 2, :],
                                r_all[32 * b:32 * b + 2, :],
                                start=True, stop=True, tile_position=(32 * b, 0))

                        si = (rep * NG + g) * SPG + q * SPQ + h2  # global slab idx
                        xn = xn_pool.tile([P, slab], comp_dt, tag="xn")
                        if fused:
                            nc.vector._custom_dve(
                                TT_MUL_MAX, out=xn[:],
                                in0=gbuf[:, qbase + h2 * slab:
                                          qbase + (h2 + 1) * slab],
                                in1=rbp[:], s0=-3.0e38,
                                accum_out=maxbuf[:, si:si + 1])
                        else:
                            nc.vector.tensor_mul(
                                xn[:],
                                gbuf[:, qbase + h2 * slab:qbase + (h2 + 1) * slab],
                                rbp[:])
                            src, dst = maccs[si % 2], maccs[1 - si % 2]
                            nc.vector.tensor_max(dst[:], src[:], xn[:])

                        for j in range(CPS):
                            cg = si * CPS + j    # global chunk idx
                            nc.tensor.matmul(
                                sacc[:], ident[:], xn[:, j * F:(j + 1) * F],
                                start=(cg == 0), stop=(cg == n_chunks_total - 1),
                                skip_group_check=True)

        # ---- finalize ----
        sum_pc = fin_pool.tile([P, 1], F32)
        s_sb = fin_pool.tile([P, F], F32)
        nc.scalar.activation(s_sb[:], sacc[:], AF.Copy)
        nc.vector.reduce_sum(sum_pc[:], s_sb[:], axis=mybir.AxisListType.X)
        max_pc = fin_pool.tile([P, 1], F32)
        if fused:
            nc.vector.reduce_max(max_pc[:], maxbuf[:], axis=mybir.AxisListType.X)
        else:
            nc.vector.memset(maxbuf[:], -2.0)
            mfin = fin_pool.tile([P, slab], comp_dt)
            nc.vector.tensor_max(mfin[:], maccs[0][:], maccs[1][:])
            nc.vector.reduce_max(max_pc[:], mfin[:], axis=mybir.AxisListType.X)

        # fold halves (partitions 64:128 -> 0:64) via SBUF->SBUF DMA realign
        hi2 = fin_pool.tile([64, 2], F32)
        nc.gpsimd.dma_start(hi2[:, 0:1], sum_pc[64:128, :])
        nc.gpsimd.dma_start(hi2[:, 1:2], max_pc[64:128, :])
        s64 = fin_pool.tile([64, 1], F32)
        nc.vector.tensor_add(s64[:], sum_pc[0:64, :], hi2[:, 0:1])
        m64 = fin_pool.tile([64, 1], F32)
        nc.vector.tensor_max(m64[:], max_pc[0:64, :], hi2[:, 1:2])
        avg = fin_pool.tile([64, 1], F32)
        nc.vector.tensor_scalar_mul(avg[:], s64[:], 1.0 / (S * repeat))
        o = fin_pool.tile([64, 1], F32)
        nc.vector.tensor_add(o[:], avg[:], m64[:])
        orelu = fin_pool.tile([64, 1], F32)
        nc.vector.tensor_scalar_max(orelu[:], o[:], 0.0)
        o2 = fin_pool.tile([64, 1], F32)
        nc.vector.tensor_mul(o2[:], orelu[:], orelu[:])
        att_s = fin_pool.tile([64, 1], F32)
        nc.scalar.activation(att_s[:], o2[:], AF.Sigmoid)
        nc.gpsimd.dma_start(y.ap(), att_s[:])
    return nc


def _split_multi_waits(nc, max_waits=1):
    """This walrus build encodes at most one sync-wait per CTRL instruction;
    hoist extra waits into single-wait NoOps placed just before."""
    for f in nc.m.functions:
        for bb in f.blocks:
            insts = list(bb.instructions)
            out = []
            changed = False
            for ins in insts:
                si = ins.sync_info
                if si is not None and si.on_wait and len(si.on_wait) > max_waits:
                    waits = list(si.on_wait)
                    for w in waits[:-max_waits]:
                        out.append(mybir.InstNoOp(
                            name=nc.get_next_instruction_name(),
                            sync_info=mybir.SyncInfo(on_wait=[w], on_update=[]),
                            bass_nofuse=True,
                            engine=ins.engine,
                        ))
                    si.on_wait = waits[-max_waits:]
                    ins.sync_info = si
                    changed = True
                out.append(ins)
            if changed:
                bb.instructions = out


def build_nc(repeat=1, **kw):
    nc = bass.Bass("TRN2", target_bir_lowering=False, debug=False,
                   num_devices=N_CORES)
    _build_kernel_body(nc, repeat=repeat, **kw)
    _split_multi_waits(nc)
    # Raw Bass skips the extended-inst lowering pass; without it InstISA
    # instructions (tensor_tensor_reduce) reach walrus with empty .instr.
    mybir.codegen_inst_isa_subclasses(nc)
    return nc


def kernel(x):
    """x: [8, 64, 32, 128, 128] f32 -> att [8, 64, 1, 1, 1] f32."""
    from concourse.bass_utils import run_bass_kernel_spmd

    x = np.ascontiguousarray(np.asarray(x, dtype=np.float32))
    assert x.shape == (B, C, D, H, W)
    nc = build_nc()
    in_maps = [{"x": x[i].reshape(C, S)} for i in range(N_CORES)]
    res = run_bass_kernel_spmd(nc, in_maps, core_ids=list(range(N_CORES)))
    att = np.stack([res.results[i]["att"].reshape(C) for i in range(N_CORES)])
    return att.reshape(B, C, 1, 1, 1).astype(np.float32)
